# revision 1
# baseline (speedup 1.0000x reference)
"""Trainium2 kernel for nn_ConvTrace: batch of 64 graphs, conv -> traces of
matrix powers -> coef-weighted sum.

Pipeline:
- Host: 6x6 conv via im2col GEMM (BLAS), zero-pad 251->256, round inputs to
  float32r (11-bit mantissa, RNE), pack natural+transposed layouts, and
  compute t2 = tr(C^2) = <C, C^T> in full precision.
- Device (8 NeuronCores, data-parallel over the batch, 64 (b,ch) pairs/core):
  per pair, two fp32r matmul products on the PE (N=256 -> 1 cyc/row):
  D = C2^T = mm(lhsT=Cn, rhs=Ct) and C3 = C2@C = mm(lhsT=ds, rhs=Cn),
  with a single PSUM->SBUF rounding copy (ds, ScalarE). Traces as
  elementwise dots: t3 = <D, C> (GpSimd product + ScalarE accumulate),
  t4 = <C3, C^T> and t5 = <C3, C2^T> (VectorE fused multiply-reduce from
  PSUM). Per-partition partials are DMA'd out.
- Host: reduce partials over partitions in float64 and apply the power/coef
  math.
"""

import os
from contextlib import ExitStack

import numpy as np

B = 64
G = 256
KK = 6
CH = 8
ROWS = 4
COLS = 3
H = G - KK + 1  # 251
NCORES = 8
PAIRS_PER_CORE = (B // NCORES) * CH  # 64

_COMPILED = None
LAST_EXEC_NS = None


def _rne_f32r(v):
    u = np.ascontiguousarray(v, dtype=np.float32).view(np.uint32).astype(np.uint64)
    u = (u + np.uint64(0x800)) & np.uint64(0xFFFFF000)
    return u.astype(np.uint32).view(np.float32)


def _build():
    """Build + compile the SPMD bass kernel once per process."""
    global _COMPILED
    if _COMPILED is not None:
        return _COMPILED

    import concourse.bacc as bacc
    import concourse.tile as tile
    from concourse import mybir

    F32 = mybir.dt.float32
    F32R = mybir.dt.float32r
    npair = PAIRS_PER_CORE

    nc = bacc.Bacc(None, target_bir_lowering=False)
    cn_d = nc.declare_dram_parameter("cn", [npair, 128, 2, 256], F32R, isOutput=False)
    ct_d = nc.declare_dram_parameter("ct", [npair, 128, 2, 256], F32R, isOutput=False)
    pa_d = nc.declare_dram_parameter("pa", [128, npair * 2], F32, isOutput=True)
    pb_d = nc.declare_dram_parameter("pb", [128, npair], F32, isOutput=True)

    with tile.TileContext(nc) as tc, ExitStack() as ctx:
        inp = ctx.enter_context(tc.tile_pool(name="inp", bufs=10))
        sb = ctx.enter_context(tc.tile_pool(name="sb", bufs=6))
        scr = ctx.enter_context(tc.tile_pool(name="scr", bufs=3))
        pp = ctx.enter_context(tc.tile_pool(name="pp", bufs=1))
        ps_d = ctx.enter_context(tc.tile_pool(name="ps_d", bufs=2, space="PSUM"))
        ps_c3 = ctx.enter_context(tc.tile_pool(name="ps_c3", bufs=4, space="PSUM"))

        partials = pp.tile([128, npair * 2], F32)
        partials_b = pp.tile([128, npair], F32)

        for pair in range(npair):
            cnt = inp.tile([128, 2, 256], F32R, tag="cn")
            ctt = inp.tile([128, 2, 256], F32R, tag="ct")
            nc.sync.dma_start(out=cnt[:], in_=cn_d[pair])
            nc.sync.dma_start(out=ctt[:], in_=ct_d[pair])
            cn = cnt[:]
            ct = ctt[:]

            def mm4(out_ps, lhs_t, rhs_t):
                # one PSUM accumulation group spanning the whole bank
                for i, (q, kt) in enumerate(((0, 0), (1, 0), (0, 1), (1, 1))):
                    nc.tensor.matmul(
                        out_ps[:, q, :],
                        lhs_t[:, kt, q * 128:(q + 1) * 128],
                        rhs_t[:, kt, :],
                        start=(i == 0),
                        stop=(i == 3),
                    )

            # D = C2^T = mm(cn, ct); C3 = C2@C = mm(ds, cn). Traces:
            # t3 = <D, C> (GpSimd+ACT), t4 = <C3, C^T> (DVE), t5 = <C3, D> (DVE)
            pd = ps_d.tile([128, 2, 256], F32)
            mm4(pd, cn, ct)
            ds = sb.tile([128, 2, 256], F32R, tag="ds")
            nc.scalar.copy(ds[:], pd[:])

            pc3 = ps_c3.tile([128, 2, 256], F32)
            mm4(pc3, ds, cn)

            def dot(col, a, b):
                out = scr.tile([128, 2, 256], F32, tag="scr")
                nc.vector.scalar_tensor_tensor(
                    out=out[:],
                    in0=a,
                    scalar=1.0,
                    in1=b,
                    op0=mybir.AluOpType.mult,
                    op1=mybir.AluOpType.mult,
                    accum_out=partials[:, col:col + 1],
                )

            ct_f = ct.bitcast(F32)
            cn_f = cn.bitcast(F32)
            # t3 = <D, C>: product on GpSimd, accumulate on ScalarE
            t3p = scr.tile([128, 2, 256], F32, tag="t3p")
            nc.gpsimd.tensor_mul(t3p[:], ds[:].bitcast(F32), cn_f)
            t3o = scr.tile([128, 2, 256], F32, tag="t3o")
            nc.scalar.activation(t3o[:], t3p[:], mybir.ActivationFunctionType.Copy,
                                 accum_out=partials_b[:, pair:pair + 1])
            dot(pair * 2 + 0, pc3[:], ct_f)                    # t4 (DVE)
            dot(pair * 2 + 1, pc3[:], ds[:].bitcast(F32))      # t5 (DVE)

        nc.sync.dma_start(out=pa_d[:], in_=partials[:])
        nc.sync.dma_start(out=pb_d[:], in_=partials_b[:])

    nc.compile()
    _COMPILED = nc
    return nc


def kernel(x, conv_w, conv_b, coef):
    global LAST_EXEC_NS
    x = np.asarray(x, dtype=np.float32)
    conv_w = np.asarray(conv_w, dtype=np.float32)
    conv_b = np.asarray(conv_b, dtype=np.float32)
    coef = np.asarray(coef, dtype=np.float32)

    # --- host: conv via im2col GEMM ---
    from numpy.lib.stride_tricks import sliding_window_view
    win = sliding_window_view(x, (KK, KK), axis=(1, 2))      # [B,H,H,KK,KK]
    patches = np.ascontiguousarray(win).reshape(B, H * H, KK * KK)
    wmat = conv_w.reshape(CH, KK * KK)
    C = patches @ wmat.T                                      # [B, H*H, CH]
    C = C.transpose(0, 2, 1).reshape(B, CH, H, H) + conv_b[None, :, None, None]

    Cpad = np.zeros((B * CH, 256, 256), np.float32)
    Cpad[:, :H, :H] = C.reshape(B * CH, H, H)

    # t2 in full precision on host (the dominant-cancellation trace)
    t2 = np.einsum("pij,pji->p", Cpad.astype(np.float64), Cpad.astype(np.float64))

    # pack rounded layouts
    Cr = _rne_f32r(Cpad)                                      # [512,256,256]
    n = B * CH
    cn = np.ascontiguousarray(Cr.reshape(n, 2, 128, 256).transpose(0, 2, 1, 3))
    ct = np.ascontiguousarray(
        Cr.transpose(0, 2, 1).reshape(n, 2, 128, 256).transpose(0, 2, 1, 3))

    nc = _build()
    from concourse.bass_utils import run_bass_kernel_spmd

    npair = PAIRS_PER_CORE
    in_maps = [
        {"cn": cn[c * npair:(c + 1) * npair], "ct": ct[c * npair:(c + 1) * npair]}
        for c in range(NCORES)
    ]

    trace = os.environ.get("CONVTRACE_PROFILE", "0") == "1"
    if trace:
        import sys
        import types
        if "antenv.axon_hooks" not in sys.modules:
            import antenv  # noqa: F401
            from trn_agent_boot.trn_boot import _ntff_profile_via_ctypes
            hook = _ntff_profile_via_ctypes("/opt/axon/libaxon_pjrt.so")
            mod = types.ModuleType("antenv.axon_hooks")
            mod.get_axon_ntff_profile_hook = lambda: hook
            mod.set_axon_ntff_profile_hook = lambda h: None
            sys.modules["antenv.axon_hooks"] = mod
        import concourse.bass_utils as bu
        bu.upload_artifacts = lambda tmpdir: tmpdir

    res = run_bass_kernel_spmd(nc, in_maps, list(range(NCORES)), trace=trace)
    LAST_EXEC_NS = res.exec_time_ns

    # --- host: finalize in float64 ---
    ts = np.empty((B * CH, 4), np.float64)
    ts[:, 0] = t2
    for c in range(NCORES):
        pa = res.results[c]["pa"].astype(np.float64)           # [128, npair*2]
        t45 = pa.sum(axis=0).reshape(npair, 2)
        ts[c * npair:(c + 1) * npair, 2] = t45[:, 0]
        ts[c * npair:(c + 1) * npair, 3] = t45[:, 1]
        ts[c * npair:(c + 1) * npair, 1] = res.results[c]["pb"].astype(np.float64).sum(axis=0)

    ts = ts.reshape(B, CH, 4)
    jpow = np.arange(1, COLS + 1, dtype=np.float64)
    retm = ts[..., None] ** jpow                               # [B,CH,ROWS,COLS]
    exps = (np.arange(ROWS, dtype=np.float64)[:, None]
            + np.arange(COLS, dtype=np.float64)[None, :] + 1.0)
    retm = retm / (np.float64(H * H) ** exps)
    out = (coef.astype(np.float64)[None] * retm).sum(axis=(1, 2, 3))
    return out.astype(np.float32)



# revision 5
# speedup vs baseline: 1.1644x; 1.1644x over previous
"""Trainium2 kernel for nn_ConvTrace: batch of 64 graphs, conv -> traces of
matrix powers -> coef-weighted sum.

Pipeline (v2, all-bf16):
- Host: 6x6 conv via im2col GEMM (BLAS), zero-pad 251->256, round to bf16,
  pack natural+transposed layouts, compute t2 = tr(C^2) exactly in float64.
- Device (8 NeuronCores, data-parallel, 64 (b,ch) pairs/core, 32 groups of
  2 pairs): per pair two bf16 PE products, D = (C^2)^T = mm(lhsT=Cn, rhs=Ct)
  and C3 = C^2@C = mm(lhsT=ds, rhs=Cn). PSUM->SBUF bf16 copies batched per
  group: ds (ScalarE) and gs=C3 (GpSimd). Traces as all-SBUF bf16 DVE dots
  (fast perf modes): t3=<ds,Cn>, t4=<gs,Ct>, t5=<gs,ds>, per-partition
  partials accumulated into one [128,192] tile, DMA'd out once.
- Host: reduce partials over partitions in float64, apply power/coef math.
"""

import os
from contextlib import ExitStack

import numpy as np
import ml_dtypes

B = 64
G = 256
KK = 6
CH = 8
ROWS = 4
COLS = 3
H = G - KK + 1  # 251
NCORES = 8
PAIRS_PER_CORE = (B // NCORES) * CH  # 64
GROUPS = PAIRS_PER_CORE // 2         # 32 groups of 2 pairs

_COMPILED = None
LAST_EXEC_NS = None

NPBF16 = ml_dtypes.bfloat16


def _build():
    """Build + compile the SPMD bass kernel once per process."""
    global _COMPILED
    if _COMPILED is not None:
        return _COMPILED

    import concourse.bacc as bacc
    import concourse.tile as tile
    from concourse import mybir

    F32 = mybir.dt.float32
    BF16 = mybir.dt.bfloat16

    nc = bacc.Bacc(None, target_bir_lowering=False)
    cn_d = nc.declare_dram_parameter("cn", [GROUPS, 128, 2, 2, 256], BF16, isOutput=False)
    ct_d = nc.declare_dram_parameter("ct", [GROUPS, 128, 2, 2, 256], BF16, isOutput=False)
    pa_d = nc.declare_dram_parameter("pa", [128, PAIRS_PER_CORE * 3], F32, isOutput=True)

    with tile.TileContext(nc) as tc, ExitStack() as ctx:
        inp = ctx.enter_context(tc.tile_pool(name="inp", bufs=4))
        dsp = ctx.enter_context(tc.tile_pool(name="dsp", bufs=3))
        gsp = ctx.enter_context(tc.tile_pool(name="gsp", bufs=3))
        scr = ctx.enter_context(tc.tile_pool(name="scr", bufs=6))
        pp = ctx.enter_context(tc.tile_pool(name="pp", bufs=1))
        ps_a = ctx.enter_context(tc.tile_pool(name="ps_a", bufs=2, space="PSUM"))
        ps_b = ctx.enter_context(tc.tile_pool(name="ps_b", bufs=2, space="PSUM"))

        partials = pp.tile([128, PAIRS_PER_CORE * 3], F32)

        def mm4(out3, lhs3, rhs3):
            # out3 [128,2,256] f32 psum; lhs3/rhs3 [128,2,256] bf16 sbuf
            # one PSUM accumulation group per pair-bank
            for i, (q, kt) in enumerate(((0, 0), (1, 0), (0, 1), (1, 1))):
                nc.tensor.matmul(
                    out3[:, q, :],
                    lhs3[:, kt, q * 128:(q + 1) * 128],
                    rhs3[:, kt, :],
                    start=(i == 0),
                    stop=(i == 3),
                )

        def dot(col, a, b, eng):
            out = scr.tile([128, 2, 256], BF16, tag="scr")
            eng.scalar_tensor_tensor(
                out=out[:],
                in0=a,
                scalar=1.0,
                in1=b,
                op0=mybir.AluOpType.mult,
                op1=mybir.AluOpType.mult,
                accum_out=partials[:, col:col + 1],
            )

        for g in range(GROUPS):
            # tuning knob: which groups get an SBUF copy of C^3 (gs);
            # the rest dot t4/t5 straight from PSUM on DVE
            has_gs = (g % 4) < 3

            cnt = inp.tile([128, 2, 2, 256], BF16, tag="cn")
            ctt = inp.tile([128, 2, 2, 256], BF16, tag="ct")
            nc.sync.dma_start(out=cnt[:], in_=cn_d[g])
            nc.sync.dma_start(out=ctt[:], in_=ct_d[g])

            pd = ps_a.tile([128, 2, 2, 256], F32, tag="pd")
            for p in range(2):
                mm4(pd[:, p], cnt[:, p], ctt[:, p])

            ds = dsp.tile([128, 2, 2, 256], BF16, tag="ds")
            nc.scalar.copy(ds[:], pd[:])

            pc3 = ps_b.tile([128, 2, 2, 256], F32, tag="pc3")
            for p in range(2):
                mm4(pc3[:, p], ds[:, p], cnt[:, p])

            if has_gs:
                gs = gsp.tile([128, 2, 2, 256], BF16, tag="gs")
                nc.scalar.copy(gs[:], pc3[:])

            for p in range(2):
                pair = g * 2 + p
                # t3 = <C^2T, C>  (always all-SBUF)
                dot(pair * 3 + 0, ds[:, p], cnt[:, p], nc.vector)
                if has_gs:
                    dot(pair * 3 + 1, gs[:, p], ctt[:, p], nc.vector)   # t4
                    dot(pair * 3 + 2, gs[:, p], ds[:, p], nc.vector)    # t5
                else:
                    dot(pair * 3 + 1, pc3[:, p], ctt[:, p], nc.vector)  # t4
                    dot(pair * 3 + 2, pc3[:, p], ds[:, p], nc.vector)   # t5

        nc.sync.dma_start(out=pa_d[:], in_=partials[:])

    nc.compile()
    _COMPILED = nc
    return nc


def kernel(x, conv_w, conv_b, coef):
    global LAST_EXEC_NS
    x = np.asarray(x, dtype=np.float32)
    conv_w = np.asarray(conv_w, dtype=np.float32)
    conv_b = np.asarray(conv_b, dtype=np.float32)
    coef = np.asarray(coef, dtype=np.float32)

    # --- host: conv via im2col GEMM ---
    from numpy.lib.stride_tricks import sliding_window_view
    win = sliding_window_view(x, (KK, KK), axis=(1, 2))      # [B,H,H,KK,KK]
    patches = np.ascontiguousarray(win).reshape(B, H * H, KK * KK)
    wmat = conv_w.reshape(CH, KK * KK)
    C = patches @ wmat.T                                      # [B, H*H, CH]
    C = C.transpose(0, 2, 1).reshape(B, CH, H, H) + conv_b[None, :, None, None]

    Cpad = np.zeros((B * CH, 256, 256), np.float32)
    Cpad[:, :H, :H] = C.reshape(B * CH, H, H)

    # t2 in full precision on host (the dominant-cancellation trace)
    t2 = np.einsum("pij,pji->p", Cpad.astype(np.float64), Cpad.astype(np.float64))

    # pack bf16 layouts: [core][group, part, pair_in_group, kt, col]
    Cb = Cpad.astype(NPBF16)                                  # [512,256,256]
    Ctb = np.ascontiguousarray(Cb.transpose(0, 2, 1))
    def pack(a):
        v = a.reshape(NCORES, GROUPS, 2, 2, 128, 256)         # c,g,pp,kt,p,j
        return np.ascontiguousarray(v.transpose(0, 1, 4, 2, 3, 5))
    cn = pack(Cb)
    ct = pack(Ctb)

    nc = _build()
    from concourse.bass_utils import run_bass_kernel_spmd

    in_maps = [{"cn": cn[c], "ct": ct[c]} for c in range(NCORES)]

    trace = os.environ.get("CONVTRACE_PROFILE", "0") == "1"
    if trace:
        import sys
        import types
        if "antenv.axon_hooks" not in sys.modules:
            import antenv  # noqa: F401
            from trn_agent_boot.trn_boot import _ntff_profile_via_ctypes
            hook = _ntff_profile_via_ctypes("/opt/axon/libaxon_pjrt.so")
            mod = types.ModuleType("antenv.axon_hooks")
            mod.get_axon_ntff_profile_hook = lambda: hook
            mod.set_axon_ntff_profile_hook = lambda h: None
            sys.modules["antenv.axon_hooks"] = mod
        import concourse.bass_utils as bu
        bu.upload_artifacts = lambda tmpdir: tmpdir

    res = run_bass_kernel_spmd(nc, in_maps, list(range(NCORES)), trace=trace)
    LAST_EXEC_NS = res.exec_time_ns

    # --- host: finalize in float64 ---
    ts = np.empty((B * CH, 4), np.float64)
    ts[:, 0] = t2
    npair = PAIRS_PER_CORE
    for c in range(NCORES):
        pa = res.results[c]["pa"].astype(np.float64)           # [128, npair*3]
        t345 = pa.sum(axis=0).reshape(npair, 3)
        ts[c * npair:(c + 1) * npair, 1:] = t345

    ts = ts.reshape(B, CH, 4)
    jpow = np.arange(1, COLS + 1, dtype=np.float64)
    retm = ts[..., None] ** jpow                               # [B,CH,ROWS,COLS]
    exps = (np.arange(ROWS, dtype=np.float64)[:, None]
            + np.arange(COLS, dtype=np.float64)[None, :] + 1.0)
    retm = retm / (np.float64(H * H) ** exps)
    out = (coef.astype(np.float64)[None] * retm).sum(axis=(1, 2, 3))
    return out.astype(np.float32)
